# revision 3
# baseline (speedup 1.0000x reference)
"""TRN2 Bass kernel for nn_CommLayer (gnn message passing).

Math: x [B=65536, 512] viewed as [B, 8 agents, 64]; per agent a:
    y_a = tanh(x_a @ Wh.T + (sum_{a'!=a} x_{a'}) @ Wc.T / 7)
Rewritten with s = sum_a x_a:
    y_a = tanh(x_a @ WdT + s @ Wc7T),  WdT = Wh.T - Wc.T/7, Wc7T = Wc.T/7
which is a block-diagonal matmul plus a shared rank-64 term -- 7x less
PE work than the dense 512x512 matmul.

Layout strategy: everything runs in the TRANSPOSED domain in fp16.
The host uploads per core xta = [x_shard.T ; s_shard.T] (shape
[576, 8192], fp16) and reads back y.T ([512, 8192], fp16), so the
device never transposes anything: plain contiguous DMAs in/out, and
every matmul's stationary operand is a small CONSTANT weight:
    yT[co*128:(co+1)*128, r] = tanh(wcs.T @ sT + wd2.T @ xT_chunk_co)
with wd2 = kron(I2, WdT) [128,128] and wcs = [Wc7T | Wc7T] [64,128].

fp16 end-to-end: halves HBM traffic vs fp32 (17 MiB/core total) and
runs the PE at 1 cycle/row. Max rel err ~4e-3 vs the f32 reference
(tolerance 2e-2).

Per 512-row group (16 groups/core): 2 input DMAs (sync queue),
8 matmuls (4x shared k=64 + 4x blockdiag k=128, all F=512), 4 tanh
activations (scalar), 1 output DMA (gpsimd queue). PSUM pool of 8
banks keeps two groups in flight.
"""
import sys

sys.path.insert(0, "/opt/trn_rl_repo")

import numpy as np

BATCH = 65536
D = 512
NAGENT = 8
DA = 64
NORM = NAGENT - 1
NCORES = 8
SHARD = BATCH // NCORES  # 8192
R = 512                  # rows per group
NGROUP = SHARD // R      # 16
NCHUNK = D // 128        # 4
XROWS = D + DA           # 576: x.T stacked with s.T

_CACHE: dict = {}


def _build_nc():
    import concourse.mybir as mybir
    import concourse.tile as tile
    from concourse import bacc

    nc = bacc.Bacc("TRN2", target_bir_lowering=False, debug=False)

    f16 = mybir.dt.float16
    f32 = mybir.dt.float32

    xt_d = nc.dram_tensor("xt", [XROWS, SHARD], f16, kind="ExternalInput")
    wd2_d = nc.dram_tensor("wd2", [128, 128], f16, kind="ExternalInput")
    wcs_d = nc.dram_tensor("wcs", [DA, 128], f16, kind="ExternalInput")
    yt_d = nc.dram_tensor("yt", [D, SHARD], f16, kind="ExternalOutput")

    xall = xt_d[:]
    xv = xall[0:D, :].rearrange("(c p) f -> p c f", p=128)   # [128, 4, 8192]
    sv = xall[D:XROWS, :]                                    # [64, 8192]
    yv = yt_d[:].rearrange("(c p) f -> p c f", p=128)        # [128, 4, 8192]

    with tile.TileContext(nc) as tc:
        with (
            tc.tile_pool(name="const", bufs=1) as const,
            tc.tile_pool(name="xg", bufs=3) as xgp,
            tc.tile_pool(name="og", bufs=3) as ogp,
            tc.tile_pool(name="psy", bufs=8, space="PSUM") as psyp,
        ):
            # weights ride the scalar queue at t=0 (idle until tanhs start)
            wd2 = const.tile([128, 128], f16)
            nc.scalar.dma_start(wd2[:], wd2_d[:])
            wcs = const.tile([DA, 128], f16)
            nc.scalar.dma_start(wcs[:], wcs_d[:])

            xg_tiles = {}

            def load_group(g, split=False):
                gs = slice(g * R, (g + 1) * R)
                xg = xgp.tile([128, NCHUNK + 1, R], f16, tag="xg", name=f"xg{g}")
                # sT first: the shared matmuls touch it before the chunks
                nc.sync.dma_start(xg[0:DA, NCHUNK, :], sv[:, gs])
                if split:
                    # per-chunk loads so chunk-co compute starts as soon as
                    # its slice lands; gpsimd helps while stores are idle
                    engs = [nc.sync, nc.gpsimd, nc.sync, nc.gpsimd]
                    for c in range(NCHUNK):
                        engs[c].dma_start(xg[:, c, :], xv[:, c, gs])
                else:
                    nc.sync.dma_start(xg[:, 0:NCHUNK, :], xv[:, :, gs])
                xg_tiles[g] = xg

            load_group(0, split=True)

            for g in range(NGROUP):
                if g + 1 < NGROUP:
                    load_group(g + 1)
                xg = xg_tiles.pop(g)
                gs = slice(g * R, (g + 1) * R)
                og = ogp.tile([128, NCHUNK, R], f16, tag="og", name=f"og{g}")
                psy = [
                    psyp.tile([128, R], f32, tag="psy", name=f"psy{g}_{co}")
                    for co in range(NCHUNK)
                ]
                # grouped by stationary: 4x wcs then 4x wd2
                for co in range(NCHUNK):
                    nc.tensor.matmul(
                        psy[co][:],
                        wcs[:],
                        xg[0:DA, NCHUNK, :],
                        start=True,
                        stop=False,
                    )
                for co in range(NCHUNK):
                    nc.tensor.matmul(
                        psy[co][:],
                        wd2[:],
                        xg[:, co, :],
                        start=False,
                        stop=True,
                    )
                for co in range(NCHUNK):
                    nc.scalar.activation(
                        og[:, co, :],
                        psy[co][:],
                        mybir.ActivationFunctionType.Tanh,
                    )
                nc.gpsimd.dma_start(yv[:, :, gs], og[:])

    nc.compile()
    return nc


def _get_nc():
    if "nc" not in _CACHE:
        _CACHE["nc"] = _build_nc()
    return _CACHE["nc"]


def _prepare_in_maps(inputs) -> list[dict]:
    """Full inputs -> per-core in_maps (host does transpose + fp16 cast)."""
    x = np.asarray(inputs["x"], dtype=np.float32)
    hw = np.asarray(inputs["hidden_weights"], dtype=np.float32)
    cw = np.asarray(inputs["communication_weights"], dtype=np.float32)
    assert x.shape == (BATCH, D), x.shape

    wc7t = cw.T / np.float32(NORM)          # [64, 64]
    wdt = hw.T - wc7t                       # [64, 64]
    wd2 = np.zeros((128, 128), dtype=np.float16)
    wd2[0:64, 0:64] = wdt
    wd2[64:128, 64:128] = wdt
    wcs = np.concatenate([wc7t, wc7t], axis=1).astype(np.float16)  # [64, 128]

    s = x.reshape(BATCH, NAGENT, DA).sum(axis=1)        # [B, 64] in f32
    x16 = x.astype(np.float16)
    s16 = s.astype(np.float16)

    in_maps = []
    for i in range(NCORES):
        rows = slice(i * SHARD, (i + 1) * SHARD)
        xta = np.empty((XROWS, SHARD), dtype=np.float16)
        xta[0:D] = x16[rows].T
        xta[D:XROWS] = s16[rows].T
        in_maps.append({"xt": xta, "wd2": wd2, "wcs": wcs})
    return in_maps


def kernel(**inputs) -> np.ndarray:
    from concourse.bass_utils import run_bass_kernel_spmd

    nc = _get_nc()
    in_maps = _prepare_in_maps(inputs)
    res = run_bass_kernel_spmd(nc, in_maps, core_ids=list(range(NCORES)))
    y = np.empty((BATCH, D), dtype=np.float32)
    for i, r in enumerate(res.results):
        y[i * SHARD:(i + 1) * SHARD] = r["yt"].T
    return y


# revision 4
# speedup vs baseline: 2.0019x; 2.0019x over previous
"""TRN2 Bass kernel for nn_CommLayer (gnn message passing).

Math: x [B=65536, 512] viewed as [B, 8 agents, 64]; per agent a:
    y_a = tanh(x_a @ Wh.T + (sum_{a'!=a} x_{a'}) @ Wc.T / 7)
Rewritten with s = sum_a x_a:
    y_a = tanh(x_a @ WdT + s @ Wc7T),  WdT = Wh.T - Wc.T/7, Wc7T = Wc.T/7
a block-diagonal matmul plus a shared rank-64 term -- 7x less PE work
than the dense 512x512 matmul.

Everything runs in the TRANSPOSED domain in fp16. The host uploads per
core a group-tiled tensor x5 [16 groups][128 part][5 slots][512 rows]:
slots 0-3 are x.T chunks, slot 4 is [s.T ; s.T] (duplicated so the
shared matmul streams a full 128-partition moving operand -- k=64
matmuls leave half the PE array idle, which keeps the HAM clock gate
at 1.2 GHz; full-width matmuls run warm at 2.4 GHz). The host reads
back y.T group-tiled and undoes the layout. All DMA partition lines
are 4-5 KiB contiguous.

Per 512-row group (16 groups/core): 1 input DMA (alternating
sync/scalar queues), 8 k=128 fp16 matmuls F=512 (4x shared with
zero-padded stationary + 4x blockdiag, all into one 4-bank PSUM tile),
1 tanh [128, 2048] (scalar), 1 output DMA (gpsimd). Max rel err ~6e-3
(tolerance 2e-2).
"""
import sys

sys.path.insert(0, "/opt/trn_rl_repo")

import numpy as np

BATCH = 65536
D = 512
NAGENT = 8
DA = 64
NORM = NAGENT - 1
NCORES = 8
SHARD = BATCH // NCORES  # 8192
R = 512                  # rows per group
NGROUP = SHARD // R      # 16
NCHUNK = D // 128        # 4
NSLOT = NCHUNK + 1       # 4 x.T chunks + 1 [s.T; s.T] slot

_CACHE: dict = {}


def _build_nc():
    import concourse.mybir as mybir
    import concourse.tile as tile
    from concourse import bacc

    nc = bacc.Bacc("TRN2", target_bir_lowering=False, debug=False)

    f16 = mybir.dt.float16
    f32 = mybir.dt.float32

    x5_d = nc.dram_tensor(
        "x5", [NGROUP * 128, NSLOT * R], f16, kind="ExternalInput"
    )
    wd2_d = nc.dram_tensor("wd2", [128, 128], f16, kind="ExternalInput")
    wcs_d = nc.dram_tensor("wcs", [128, 128], f16, kind="ExternalInput")
    y5_d = nc.dram_tensor(
        "y5", [NGROUP * 128, NCHUNK * R], f16, kind="ExternalOutput"
    )

    xv = x5_d[:].rearrange("(g p) f -> g p f", p=128)  # [16, 128, 2560]
    yv = y5_d[:].rearrange("(g p) f -> g p f", p=128)  # [16, 128, 2048]

    with tile.TileContext(nc) as tc:
        with (
            tc.tile_pool(name="const", bufs=1) as const,
            tc.tile_pool(name="xg", bufs=4) as xgp,
            tc.tile_pool(name="og", bufs=3) as ogp,
            tc.tile_pool(name="psy", bufs=2, space="PSUM") as psyp,
        ):
            wd2 = const.tile([128, 128], f16)
            nc.scalar.dma_start(wd2[:], wd2_d[:])
            wcs = const.tile([128, 128], f16)
            nc.scalar.dma_start(wcs[:], wcs_d[:])

            xg_tiles = {}

            def load_group(g):
                xg = xgp.tile([128, NSLOT * R], f16, tag="xg", name=f"xg{g}")
                eng = nc.sync if g % 2 == 0 else nc.scalar
                eng.dma_start(xg[:], xv[g])
                xg_tiles[g] = xg

            load_group(0)
            load_group(1)

            for g in range(NGROUP):
                if g + 2 < NGROUP:
                    load_group(g + 2)
                xg = xg_tiles.pop(g)
                og = ogp.tile([128, NCHUNK * R], f16, tag="og", name=f"og{g}")
                psy = psyp.tile([128, NCHUNK * R], f32, tag="psy",
                                name=f"psy{g}")
                sslot = xg[:, NCHUNK * R:NSLOT * R]
                for co in range(NCHUNK):
                    cs = slice(co * R, (co + 1) * R)
                    nc.tensor.matmul(
                        psy[:, cs], wcs[:], sslot, start=True, stop=False
                    )
                    nc.tensor.matmul(
                        psy[:, cs], wd2[:], xg[:, cs], start=False, stop=True
                    )
                nc.scalar.activation(
                    og[:], psy[:], mybir.ActivationFunctionType.Tanh
                )
                nc.gpsimd.dma_start(yv[g], og[:])

    nc.compile()
    return nc


def _get_nc():
    if "nc" not in _CACHE:
        _CACHE["nc"] = _build_nc()
    return _CACHE["nc"]


def _prepare_in_maps(inputs) -> list[dict]:
    """Full inputs -> per-core in_maps (host does transpose + fp16 cast)."""
    x = np.asarray(inputs["x"], dtype=np.float32)
    hw = np.asarray(inputs["hidden_weights"], dtype=np.float32)
    cw = np.asarray(inputs["communication_weights"], dtype=np.float32)
    assert x.shape == (BATCH, D), x.shape

    wc7t = cw.T / np.float32(NORM)          # [64, 64]
    wdt = hw.T - wc7t                       # [64, 64]
    wd2 = np.zeros((128, 128), dtype=np.float16)
    wd2[0:64, 0:64] = wdt
    wd2[64:128, 64:128] = wdt
    wcs = np.zeros((128, 128), dtype=np.float16)
    wcs[0:64, 0:64] = wc7t
    wcs[0:64, 64:128] = wc7t

    s = x.reshape(BATCH, NAGENT, DA).sum(axis=1)        # [B, 64] in f32
    x16 = x.astype(np.float16)
    s16 = s.astype(np.float16)

    in_maps = []
    for i in range(NCORES):
        rows = slice(i * SHARD, (i + 1) * SHARD)
        xt = x16[rows].T                                 # [512, 8192]
        st = s16[rows].T                                 # [64, 8192]
        # [4, 128, 16, 512] -> [16, 128, 4, 512]
        xc = np.ascontiguousarray(
            xt.reshape(NCHUNK, 128, NGROUP, R).transpose(2, 1, 0, 3)
        )
        st2 = np.concatenate([st, st], axis=0)           # [128, 8192]
        sc = np.ascontiguousarray(
            st2.reshape(128, NGROUP, R).transpose(1, 0, 2)
        )[:, :, None, :]                                 # [16, 128, 1, 512]
        x5 = np.concatenate([xc, sc], axis=2).reshape(
            NGROUP * 128, NSLOT * R
        )
        in_maps.append({"x5": x5, "wd2": wd2, "wcs": wcs})
    return in_maps


def _decode_out(res) -> np.ndarray:
    y = np.empty((BATCH, D), dtype=np.float32)
    for i, r in enumerate(res.results):
        y5 = r["y5"].reshape(NGROUP, 128, NCHUNK, R)
        # y5[g, p, co, r] = y[g*R + r, co*128 + p]
        yi = y5.transpose(0, 3, 2, 1).reshape(SHARD, D)
        y[i * SHARD:(i + 1) * SHARD] = yi
    return y


def kernel(**inputs) -> np.ndarray:
    from concourse.bass_utils import run_bass_kernel_spmd

    nc = _get_nc()
    in_maps = _prepare_in_maps(inputs)
    res = run_bass_kernel_spmd(nc, in_maps, core_ids=list(range(NCORES)))
    return _decode_out(res)


# revision 5
# speedup vs baseline: 2.1313x; 1.0647x over previous
"""TRN2 Bass kernel for nn_CommLayer (gnn message passing).

Math: x [B=65536, 512] viewed as [B, 8 agents, 64]; per agent a:
    y_a = tanh(x_a @ Wh.T + (sum_{a'!=a} x_{a'}) @ Wc.T / 7)
Rewritten with s = sum_a x_a:
    y_a = tanh(x_a @ WdT + s @ Wc7T),  WdT = Wh.T - Wc.T/7, Wc7T = Wc.T/7
a block-diagonal matmul plus a shared rank-64 term -- 7x less PE work
than the dense 512x512 matmul.

Everything runs in the TRANSPOSED domain in fp16 (max rel err ~6e-3 vs
tolerance 2e-2; bf16 fails at 4.7e-2). The host uploads per core
group-tiled x.T chunks and s.T, and reads back y.T group-tiled, so the
device never transposes: all DMA partition lines are 2-8 KiB
contiguous. fp16 halves HBM traffic (17 MiB/core total).

Engine plan per 1024-row group (8 groups/core):
  sync   : 2 input DMAs (x.T chunks 1.05 MB + s.T 0.13 MB)
  vector : duplicate s.T into partitions 64-127 (keeps every matmul
           k=128 -- half-array k=64 matmuls hold the PE's HAM clock
           gate at 1.2 GHz; full-width runs warm at 2.4 GHz)
  tensor : per chunk co: 2x(shared + blockdiag) fp16 matmuls F=512
           into a 2-bank PSUM tile; stationaries are constant weights
  scalar : 1 tanh [128, 1024] per chunk, PSUM -> fp16 SBUF
  gpsimd : 1 output DMA (1.05 MB)
PSUM: 4 tiles x 2 banks = all 8 banks, so the PE runs a full group
ahead of the activations.
"""
import sys

sys.path.insert(0, "/opt/trn_rl_repo")

import numpy as np

BATCH = 65536
D = 512
NAGENT = 8
DA = 64
NORM = NAGENT - 1
NCORES = 8
SHARD = BATCH // NCORES  # 8192
R = 1024                 # rows per group
NGROUP = SHARD // R      # 8
NCHUNK = D // 128        # 4
NMM = R // 512           # matmul F=512 slices per chunk

_CACHE: dict = {}


def _build_nc():
    import concourse.mybir as mybir
    import concourse.tile as tile
    from concourse import bacc

    nc = bacc.Bacc("TRN2", target_bir_lowering=False, debug=False)

    f16 = mybir.dt.float16
    f32 = mybir.dt.float32

    x4_d = nc.dram_tensor(
        "x4", [NGROUP * 128, NCHUNK * R], f16, kind="ExternalInput"
    )
    st_d = nc.dram_tensor("st", [NGROUP * DA, R], f16, kind="ExternalInput")
    wd2_d = nc.dram_tensor("wd2", [128, 128], f16, kind="ExternalInput")
    wcs_d = nc.dram_tensor("wcs", [128, 128], f16, kind="ExternalInput")
    y4_d = nc.dram_tensor(
        "y4", [NGROUP * 128, NCHUNK * R], f16, kind="ExternalOutput"
    )

    xv = x4_d[:].rearrange("(g p) f -> g p f", p=128)  # [8, 128, 4096]
    sv = st_d[:].rearrange("(g p) f -> g p f", p=DA)   # [8, 64, 1024]
    yv = y4_d[:].rearrange("(g p) f -> g p f", p=128)  # [8, 128, 4096]

    with tile.TileContext(nc) as tc:
        with (
            tc.tile_pool(name="const", bufs=1) as const,
            tc.tile_pool(name="xg", bufs=3) as xgp,
            tc.tile_pool(name="sg", bufs=3) as sgp,
            tc.tile_pool(name="og", bufs=3) as ogp,
            tc.tile_pool(name="psy", bufs=4, space="PSUM") as psyp,
        ):
            wd2 = const.tile([128, 128], f16)
            nc.scalar.dma_start(wd2[:], wd2_d[:])
            wcs = const.tile([128, 128], f16)
            nc.scalar.dma_start(wcs[:], wcs_d[:])

            xg_tiles = {}

            def load_group(g):
                xg = xgp.tile([128, NCHUNK * R], f16, tag="xg", name=f"xg{g}")
                sg = sgp.tile([128, R], f16, tag="sg", name=f"sg{g}")
                nc.sync.dma_start(sg[0:DA, :], sv[g])
                nc.sync.dma_start(xg[:], xv[g])
                # duplicate s.T into the upper partition half (DVE is idle;
                # keeps the shared matmuls streaming all 128 partitions)
                nc.vector.tensor_copy(sg[DA:128, :], sg[0:DA, :])
                xg_tiles[g] = (xg, sg)

            load_group(0)
            load_group(1)

            for g in range(NGROUP):
                if g + 2 < NGROUP:
                    load_group(g + 2)
                xg, sg = xg_tiles.pop(g)
                og = ogp.tile([128, NCHUNK * R], f16, tag="og", name=f"og{g}")
                for co in range(NCHUNK):
                    psy = psyp.tile([128, R], f32, tag="psy",
                                    name=f"psy{g}_{co}")
                    for h in range(NMM):
                        hs = slice(h * 512, (h + 1) * 512)
                        nc.tensor.matmul(
                            psy[:, hs], wcs[:], sg[:, hs],
                            start=True, stop=False,
                        )
                        nc.tensor.matmul(
                            psy[:, hs], wd2[:],
                            xg[:, co * R + h * 512:co * R + (h + 1) * 512],
                            start=False, stop=True,
                        )
                    nc.scalar.activation(
                        og[:, co * R:(co + 1) * R], psy[:],
                        mybir.ActivationFunctionType.Tanh,
                    )
                nc.gpsimd.dma_start(yv[g], og[:])

    nc.compile()
    return nc


def _get_nc():
    if "nc" not in _CACHE:
        _CACHE["nc"] = _build_nc()
    return _CACHE["nc"]


def _prepare_in_maps(inputs) -> list[dict]:
    """Full inputs -> per-core in_maps (host does transpose + fp16 cast)."""
    x = np.asarray(inputs["x"], dtype=np.float32)
    hw = np.asarray(inputs["hidden_weights"], dtype=np.float32)
    cw = np.asarray(inputs["communication_weights"], dtype=np.float32)
    assert x.shape == (BATCH, D), x.shape

    wc7t = cw.T / np.float32(NORM)          # [64, 64]
    wdt = hw.T - wc7t                       # [64, 64]
    wd2 = np.zeros((128, 128), dtype=np.float16)
    wd2[0:64, 0:64] = wdt
    wd2[64:128, 64:128] = wdt
    wcs = np.zeros((128, 128), dtype=np.float16)
    wcs[0:64, 0:64] = wc7t
    wcs[0:64, 64:128] = wc7t

    s = x.reshape(BATCH, NAGENT, DA).sum(axis=1)        # [B, 64] in f32
    x16 = x.astype(np.float16)
    s16 = s.astype(np.float16)

    in_maps = []
    for i in range(NCORES):
        rows = slice(i * SHARD, (i + 1) * SHARD)
        xt = x16[rows].T                                 # [512, 8192]
        st = s16[rows].T                                 # [64, 8192]
        # [4, 128, 8, 1024] -> [8, 128, 4, 1024]
        x4 = np.ascontiguousarray(
            xt.reshape(NCHUNK, 128, NGROUP, R).transpose(2, 1, 0, 3)
        ).reshape(NGROUP * 128, NCHUNK * R)
        stg = np.ascontiguousarray(
            st.reshape(DA, NGROUP, R).transpose(1, 0, 2)
        ).reshape(NGROUP * DA, R)
        in_maps.append({"x4": x4, "st": stg, "wd2": wd2, "wcs": wcs})
    return in_maps


def _decode_out(res) -> np.ndarray:
    y = np.empty((BATCH, D), dtype=np.float32)
    for i, r in enumerate(res.results):
        y4 = r["y4"].reshape(NGROUP, 128, NCHUNK, R)
        # y4[g, p, co, r] = y[g*R + r, co*128 + p]
        yi = y4.transpose(0, 3, 2, 1).reshape(SHARD, D)
        y[i * SHARD:(i + 1) * SHARD] = yi
    return y


def kernel(**inputs) -> np.ndarray:
    from concourse.bass_utils import run_bass_kernel_spmd

    nc = _get_nc()
    in_maps = _prepare_in_maps(inputs)
    res = run_bass_kernel_spmd(nc, in_maps, core_ids=list(range(NCORES)))
    return _decode_out(res)


# revision 6
# speedup vs baseline: 2.3896x; 1.1212x over previous
"""TRN2 Bass kernel for nn_CommLayer (gnn message passing).

Math: x [B=65536, 512] viewed as [B, 8 agents, 64]; per agent a:
    y_a = tanh(x_a @ Wh.T + (sum_{a'!=a} x_{a'}) @ Wc.T / 7)
Rewritten with s = sum_a x_a:
    y_a = tanh(x_a @ WdT + s @ Wc7T),  WdT = Wh.T - Wc.T/7, Wc7T = Wc.T/7
a block-diagonal matmul plus a shared rank-64 term -- 7x less PE work
than the dense 512x512 matmul.

Everything runs in the TRANSPOSED domain in fp16 (max rel err ~6e-3 vs
tolerance 2e-2; bf16 fails at 4.7e-2). The host uploads per core
group-tiled x.T chunks and s.T, and reads back y.T group-tiled, so the
device never transposes: all DMA partition lines are 2-8 KiB
contiguous. fp16 halves HBM traffic (17 MiB/core total).

Engine plan per 1024-row group (8 groups/core):
  sync   : 2 input DMAs (x.T chunks 1.05 MB + s.T 0.13 MB)
  vector : duplicate s.T into partitions 64-127 (keeps every matmul
           k=128 -- half-array k=64 matmuls hold the PE's HAM clock
           gate at 1.2 GHz; full-width runs warm at 2.4 GHz)
  tensor : per chunk co: 2x(shared + blockdiag) fp16 matmuls F=512
           into a 2-bank PSUM tile; stationaries are constant weights
  scalar : 1 tanh [128, 1024] per chunk, PSUM -> fp16 SBUF
  gpsimd : 1 output DMA (1.05 MB)
PSUM: 4 tiles x 2 banks = all 8 banks, so the PE runs a full group
ahead of the activations.
"""
import sys

sys.path.insert(0, "/opt/trn_rl_repo")

import numpy as np

BATCH = 65536
D = 512
NAGENT = 8
DA = 64
NORM = NAGENT - 1
NCORES = 8
SHARD = BATCH // NCORES  # 8192
R = 1024                 # rows per group
NGROUP = SHARD // R      # 8
NCHUNK = D // 128        # 4
NMM = R // 512           # matmul F=512 slices per chunk

_CACHE: dict = {}


def _build_nc():
    import concourse.mybir as mybir
    import concourse.tile as tile
    from concourse import bacc

    nc = bacc.Bacc("TRN2", target_bir_lowering=False, debug=False)

    f16 = mybir.dt.float16
    f32 = mybir.dt.float32

    x4_d = nc.dram_tensor(
        "x4", [NGROUP * 128, NCHUNK * R], f16, kind="ExternalInput"
    )
    st_d = nc.dram_tensor("st", [NGROUP * DA, R], f16, kind="ExternalInput")
    wd2_d = nc.dram_tensor("wd2", [128, 128], f16, kind="ExternalInput")
    wcs_d = nc.dram_tensor("wcs", [128, 128], f16, kind="ExternalInput")
    y4_d = nc.dram_tensor(
        "y4", [NGROUP * 128, NCHUNK * R], f16, kind="ExternalOutput"
    )

    xv = x4_d[:].rearrange("(g p) f -> g p f", p=128)  # [8, 128, 4096]
    sv = st_d[:].rearrange("(g p) f -> g p f", p=DA)   # [8, 64, 1024]
    yv = y4_d[:].rearrange("(g p) f -> g p f", p=128)  # [8, 128, 4096]

    with tile.TileContext(nc) as tc:
        with (
            tc.tile_pool(name="const", bufs=1) as const,
            tc.tile_pool(name="xg", bufs=3) as xgp,
            tc.tile_pool(name="sg", bufs=3) as sgp,
            tc.tile_pool(name="og", bufs=3) as ogp,
            tc.tile_pool(name="psy", bufs=4, space="PSUM") as psyp,
        ):
            xg_tiles = {}
            HALF = NCHUNK * R // 2

            def load_group(g, split=False):
                xg = xgp.tile([128, NCHUNK * R], f16, tag="xg", name=f"xg{g}")
                sg = sgp.tile([128, R], f16, tag="sg", name=f"sg{g}")
                if split:
                    # chunk-granular arrival so the first matmuls start
                    # as soon as chunk 0 lands
                    for c in range(NCHUNK):
                        nc.sync.dma_start(
                            xg[:, c * R:(c + 1) * R],
                            xv[g][:, c * R:(c + 1) * R],
                        )
                else:
                    nc.sync.dma_start(xg[:, 0:HALF], xv[g][:, 0:HALF])
                    nc.sync.dma_start(xg[:, HALF:], xv[g][:, HALF:])
                # s.T rides the store queue (balances the two HBM streams)
                nc.gpsimd.dma_start(sg[0:DA, :], sv[g])
                # duplicate s.T into the upper partition half (DVE is idle;
                # keeps the shared matmuls streaming all 128 partitions)
                nc.vector.tensor_copy(sg[DA:128, :], sg[0:DA, :])
                xg_tiles[g] = (xg, sg)

            # input loads first in program order: their DMA queues spin up
            # ~2.5us into the NEFF preamble, before the engine barrier ends
            load_group(0, split=True)
            load_group(1)
            wd2 = const.tile([128, 128], f16)
            nc.scalar.dma_start(wd2[:], wd2_d[:])
            wcs = const.tile([128, 128], f16)
            nc.scalar.dma_start(wcs[:], wcs_d[:])

            for g in range(NGROUP):
                if g + 2 < NGROUP:
                    load_group(g + 2)
                xg, sg = xg_tiles.pop(g)
                og = ogp.tile([128, NCHUNK * R], f16, tag="og", name=f"og{g}")
                for co in range(NCHUNK):
                    psy = psyp.tile([128, R], f32, tag="psy",
                                    name=f"psy{g}_{co}")
                    for h in range(NMM):
                        hs = slice(h * 512, (h + 1) * 512)
                        nc.tensor.matmul(
                            psy[:, hs], wcs[:], sg[:, hs],
                            start=True, stop=False,
                        )
                        nc.tensor.matmul(
                            psy[:, hs], wd2[:],
                            xg[:, co * R + h * 512:co * R + (h + 1) * 512],
                            start=False, stop=True,
                        )
                    nc.scalar.activation(
                        og[:, co * R:(co + 1) * R], psy[:],
                        mybir.ActivationFunctionType.Tanh,
                    )
                    # half-group stores, alternating queues: drains the
                    # tail as tanhs retire instead of one burst at the end
                    if co == 1:
                        nc.gpsimd.dma_start(
                            yv[g][:, 0:HALF], og[:, 0:HALF]
                        )
                    elif co == 3:
                        nc.scalar.dma_start(
                            yv[g][:, HALF:], og[:, HALF:]
                        )

    nc.compile()
    return nc


def _get_nc():
    if "nc" not in _CACHE:
        _CACHE["nc"] = _build_nc()
    return _CACHE["nc"]


def _prepare_in_maps(inputs) -> list[dict]:
    """Full inputs -> per-core in_maps (host does transpose + fp16 cast)."""
    x = np.asarray(inputs["x"], dtype=np.float32)
    hw = np.asarray(inputs["hidden_weights"], dtype=np.float32)
    cw = np.asarray(inputs["communication_weights"], dtype=np.float32)
    assert x.shape == (BATCH, D), x.shape

    wc7t = cw.T / np.float32(NORM)          # [64, 64]
    wdt = hw.T - wc7t                       # [64, 64]
    wd2 = np.zeros((128, 128), dtype=np.float16)
    wd2[0:64, 0:64] = wdt
    wd2[64:128, 64:128] = wdt
    wcs = np.zeros((128, 128), dtype=np.float16)
    wcs[0:64, 0:64] = wc7t
    wcs[0:64, 64:128] = wc7t

    s = x.reshape(BATCH, NAGENT, DA).sum(axis=1)        # [B, 64] in f32
    x16 = x.astype(np.float16)
    s16 = s.astype(np.float16)

    in_maps = []
    for i in range(NCORES):
        rows = slice(i * SHARD, (i + 1) * SHARD)
        xt = x16[rows].T                                 # [512, 8192]
        st = s16[rows].T                                 # [64, 8192]
        # [4, 128, 8, 1024] -> [8, 128, 4, 1024]
        x4 = np.ascontiguousarray(
            xt.reshape(NCHUNK, 128, NGROUP, R).transpose(2, 1, 0, 3)
        ).reshape(NGROUP * 128, NCHUNK * R)
        stg = np.ascontiguousarray(
            st.reshape(DA, NGROUP, R).transpose(1, 0, 2)
        ).reshape(NGROUP * DA, R)
        in_maps.append({"x4": x4, "st": stg, "wd2": wd2, "wcs": wcs})
    return in_maps


def _decode_out(res) -> np.ndarray:
    y = np.empty((BATCH, D), dtype=np.float32)
    for i, r in enumerate(res.results):
        y4 = r["y4"].reshape(NGROUP, 128, NCHUNK, R)
        # y4[g, p, co, r] = y[g*R + r, co*128 + p]
        yi = y4.transpose(0, 3, 2, 1).reshape(SHARD, D)
        y[i * SHARD:(i + 1) * SHARD] = yi
    return y


def kernel(**inputs) -> np.ndarray:
    from concourse.bass_utils import run_bass_kernel_spmd

    nc = _get_nc()
    in_maps = _prepare_in_maps(inputs)
    res = run_bass_kernel_spmd(nc, in_maps, core_ids=list(range(NCORES)))
    return _decode_out(res)
